# revision 7
# baseline (speedup 1.0000x reference)
"""Trainium2 Bass kernel for nn_ConcatDecoder.

reference:
    norms = ||z_i||;  cos[i,j] = <z_i,z_j>/max(norms_i*norms_j, eps)
    adj[i,j,:] = concat(z[i], z[j], raw_adj[:, i, j])   -> [N, N, 2F+K]
    returns (adj, cos.reshape(N*N, 1))

Strategy: shard rows i across 8 cores (128 rows each). Each core gets the
full z (tiny) plus its raw_adj[:, i0:i0+128, :] slab, and writes its
adj[i0:i0+128] slab + cos rows.

The adj slab (136 MB/core) is written as 128 DMAs of one full row-block
adj[i] = [1024, 260] each — 1.04 MB contiguous in DRAM, sourced from an
SBUF tile [128 parts, 2080] whose partition p holds rows j=8p..8p+7.
Per i, only the z_i columns (PE K=1 broadcast + copy) and the raw columns
(pre-transposed slab slice) change; the z_j columns are prefilled once per
ring buffer. This keeps every DMA 2-D with 8320 B contiguous runs — near
the HBM write roofline — and needs no stride-0 or >2-D DMA patterns.

cos is one 128x128 @ 128x1024 PE matmul on pre-normalized rows.
"""

import sys

sys.path.insert(0, "/opt/trn_rl_repo")

import numpy as np

N, F, K, NCORES = 1024, 128, 4, 8
R = N // NCORES  # 128 rows per core
D = 2 * F + K  # 260
NB = 6  # zcat ring depth

_PROGRAM = None


def _build_program():
    import concourse.bacc as bacc
    import concourse.tile as tile
    from concourse import mybir
    from concourse.masks import make_identity

    f32 = mybir.dt.float32
    nc = bacc.Bacc("TRN2", target_bir_lowering=False, debug=False,
                   num_devices=NCORES)

    z_ext = nc.dram_tensor("z", [N, F], f32, kind="ExternalInput").ap()
    zc_ext = nc.dram_tensor("zc", [R, F], f32, kind="ExternalInput").ap()
    raw_ext = nc.dram_tensor("raw", [K, R, N], f32, kind="ExternalInput").ap()
    adj_ext = nc.dram_tensor("adj", [R, N, D], f32, kind="ExternalOutput").ap()
    cos_ext = nc.dram_tensor("cos", [R, N], f32, kind="ExternalOutput").ap()

    with tile.TileContext(nc) as tc:
        with tc.tile_pool(name="s", bufs=1) as sp, \
             tc.tile_pool(name="zc_pool", bufs=1) as zp, \
             tc.tile_pool(name="psb", bufs=4, space="PSUM") as pb, \
             tc.tile_pool(name="psw", bufs=2, space="PSUM") as pw:
            # ---- loads ----
            zct = sp.tile([R, F], f32)  # partition il = z row i0+il
            nc.sync.dma_start(zct[:], zc_ext[:])
            # zch: partition p = z rows 8p..8p+7 (j-chunk layout)
            zch = sp.tile([128, 8, F], f32)
            nc.sync.dma_start(zch[:], z_ext[:].rearrange("(p q) f -> p q f", p=128))
            # zrows: partition p, block t = z row 128t+p (for norms / znT)
            zrows = sp.tile([128, 8, F], f32)
            nc.sync.dma_start(zrows[:], z_ext[:].rearrange("(t p) f -> p t f", t=8))
            # raw slab: partition il = i, free (k, j)
            rawk = sp.tile([R, K, N], f32)
            nc.scalar.dma_start(rawk[:], raw_ext[:].rearrange("k i j -> i k j"))

            ident = sp.tile([128, 128], f32)
            make_identity(nc, ident)

            # rawZ[p, il, q, k] = raw_adj[k, i0+il, 8p+q] : 32 PE transposes
            rawZ = sp.tile([128, R, 8, K], f32)
            for k in range(K):
                for q in range(8):
                    pt = pb.tile([128, 128], f32, tag="ps128")
                    nc.tensor.transpose(
                        pt[:], rawk[:, k, :].rearrange("i (p q) -> i q p", q=8)[:, q, :],
                        ident[:])
                    nc.vector.tensor_copy(rawZ[:, :, q, k], pt[:])

            # ---- cos: normalize all rows, transpose, matmul ----
            sq = sp.tile([128, 8, F], f32)
            nc.vector.tensor_mul(sq[:], zrows[:], zrows[:])
            n2 = sp.tile([128, 8], f32)
            nc.vector.reduce_sum(n2[:], sq[:], axis=mybir.AxisListType.X)
            nrm = sp.tile([128, 8], f32)
            nc.scalar.activation(nrm[:], n2[:], mybir.ActivationFunctionType.Sqrt)
            rn = sp.tile([128, 8], f32)
            nc.vector.reciprocal(rn[:], nrm[:])
            zn = sp.tile([128, 8, F], f32)
            for t in range(8):
                nc.vector.tensor_scalar_mul(zn[:, t, :], zrows[:, t, :],
                                            rn[:, t:t + 1])
            znT = sp.tile([128, N], f32)
            for t in range(8):
                ps = pb.tile([128, 128], f32, tag="ps128")
                nc.tensor.transpose(ps[:], zn[:, t, :], ident[:])
                nc.vector.tensor_copy(znT[:, t * 128:(t + 1) * 128], ps[:])

            sqc = sp.tile([R, F], f32)
            nc.vector.tensor_mul(sqc[:], zct[:], zct[:])
            n2c = sp.tile([R, 1], f32)
            nc.vector.reduce_sum(n2c[:], sqc[:], axis=mybir.AxisListType.X)
            nrmc = sp.tile([R, 1], f32)
            nc.scalar.activation(nrmc[:], n2c[:], mybir.ActivationFunctionType.Sqrt)
            rnc = sp.tile([R, 1], f32)
            nc.vector.reciprocal(rnc[:], nrmc[:])
            zcn = sp.tile([R, F], f32)
            nc.vector.tensor_scalar_mul(zcn[:], zct[:], rnc[:])
            zcnT_ps = pb.tile([128, 128], f32, tag="ps128")
            nc.tensor.transpose(zcnT_ps[:], zcn[:], ident[:])
            zcnT = sp.tile([128, 128], f32)
            nc.vector.tensor_copy(zcnT[:], zcnT_ps[:])

            cos_sb = sp.tile([R, N], f32)
            for h in range(2):
                pc = pw.tile([128, 512], f32)
                nc.tensor.matmul(pc[:], zcnT[:], znT[:, h * 512:(h + 1) * 512])
                nc.vector.tensor_copy(cos_sb[:, h * 512:(h + 1) * 512], pc[:])
            nc.sync.dma_start(cos_ext[:], cos_sb[:])

            # ---- adj row-blocks ----
            # ring of NB persistent tiles; z_j columns prefilled once
            zcats = []
            for b in range(NB):
                zcat = zp.tile([128, 8, D], f32, tag=f"zcat{b}")
                nc.vector.tensor_copy(zcat[:, :, F:2 * F], zch[:])
                zcats.append(zcat)

            for il in range(R):
                zcat = zcats[il % NB]
                # z_i broadcast across partitions: out[p,f] = sum_k 1[k=il]*z[k,f]
                bc = pb.tile([128, 128], f32, tag="ps128")
                nc.tensor.matmul(bc[:], ident[:, il:il + 1].broadcast_to((128, 128)),
                                 zct[:])
                nc.vector.tensor_copy(
                    zcat[:, :, 0:F],
                    bc[:].unsqueeze(1).broadcast_to((128, 8, F)))
                nc.gpsimd.tensor_copy(zcat[:, :, 2 * F:D], rawZ[:, il, :, :])
                eng = nc.sync if il % 2 == 0 else nc.scalar
                eng.dma_start(
                    adj_ext[il].rearrange("(p q) d -> p q d", p=128), zcat[:])

    nc.compile()
    return nc


def _get_program():
    global _PROGRAM
    if _PROGRAM is None:
        _PROGRAM = _build_program()
    return _PROGRAM


def _make_in_maps(z, raw_adj):
    in_maps = []
    for c in range(NCORES):
        i0 = c * R
        in_maps.append({
            "z": z,
            "zc": np.ascontiguousarray(z[i0:i0 + R]),
            "raw": np.ascontiguousarray(raw_adj[:, i0:i0 + R, :]),
        })
    return in_maps


def _gather(results):
    adj = np.concatenate([results[c]["adj"] for c in range(NCORES)], axis=0)
    cos = np.concatenate([results[c]["cos"] for c in range(NCORES)],
                         axis=0).reshape(N * N, 1)
    return adj, cos


def run_traced(z, raw_adj, trace=True, tmpdir=None):
    """Run on hardware with NTFF tracing; returns ((adj, cos), results_obj)."""
    from concourse.bass_utils import run_bass_kernel_spmd

    z = np.asarray(z, dtype=np.float32)
    raw_adj = np.asarray(raw_adj, dtype=np.float32)
    nc = _get_program()
    res = run_bass_kernel_spmd(nc, _make_in_maps(z, raw_adj),
                               list(range(NCORES)), trace=trace,
                               tmpdir=tmpdir)
    return _gather(res.results), res


def kernel(z, raw_adj):
    (adj, cos), _ = run_traced(z, raw_adj, trace=False)
    return adj, cos
